# revision 2
# baseline (speedup 1.0000x reference)
"""Polynomial flow regularizer loss on 8 Trainium2 NeuronCores.

reference semantics: fit a quadratic polynomial surface (basis
[1, x, y, x^2, x*y, y^2] over a [-1,1]^2 grid) to each (b, c) image of
flow_field (64, 2, 512, 512) via least squares, and return
mean_b(sum_c(mean_pixels((f - fit)^2))).

Math: with Phi the (N, 6) basis, G = Phi^T Phi and r = Phi^T f, the
residual energy is ||f||^2 - r^T G^-1 r.  The basis is separable, so r
comes from V[a, w] = sum_h y_h^a f[h, w] (a = 0..2) via the x-side
contraction on host.  Only the GLOBAL sum of squares matters (every
(b, c) image has equal weight 1/(N*B)).

Device strategy (data-parallel over batch; core k takes 16 images):
  - Each image is 4 sub-rows of (128, 512): h = 128 t + p.
  - The 64 (img, t) units per core are split between engines and the
    host packs each engine's units contiguously in its own DRAM region
    with its own dtype:
      regA  fp8  -> ScalarE  Square + accum_out (one pass)
      reg8  fp8  -> DVE tensor_mul (1x) -> scr bf16 -> PE ones-matmul
      reg16 bf16 -> DVE tensor_mul (2x) -> scr bf16 -> PE ones-matmul
    fp8 halves HBM bytes (loss tolerates it: measured rel err ~7e-4 vs
    the 2e-2 gate); bf16 for part of DVE's share buys its 2x mode back.
  - V: per image, 4 accumulating matmuls (lhsT = y-basis chunk (128, 3)
    in the unit's dtype).  Images are spread over PE column groups
    (tile_position=(0, 32j), j = i % 4) so up to 4 chains overlap, and
    over 4 PSUM banks (g = i // 4).  PSUM exits via junk-inclusive
    whole-bank copies [0:99, :] (cost = free size, not partitions),
    alternating ScalarE / DVE, then DMA of the 12 useful rows.
  - ones-matmul reduce: lhsT = ones (128, 1) bf16, rhs = scr blocks,
    4 accumulation chains in psum bank 5 rows {0, 32, 64, 96}; exits
    via one ScalarE identity-activation with accum_out (row sums).
Host: r = V @ Xb, per-image Gram from the quantized basis (dtype mix
per image), loss = (sum sq - sum fit)/(N*B).
"""

import sys

import numpy as np

sys.path.insert(0, "/opt/trn_rl_repo")

import concourse.bacc as bacc
import concourse.bass as bass
import concourse.tile as tile
from concourse import mybir
from concourse.bass_utils import run_bass_kernel_spmd

B, C, H, W = 64, 2, 512, 512
N_CORES = 8
IMGS = (B // N_CORES) * C  # 16 images per core
T = 4  # sub-rows per image, h = 128 t + p
N_UNITS = IMGS * T  # 64
F32 = mybir.dt.float32
BF16 = mybir.dt.bfloat16
FP8 = mybir.dt.float8e4

# unit counts per engine-region: ACT(fp8), DVE(fp8), DVE(bf16)
NA, N8, N16 = 30, 15, 19
CHUNKS = [2, 2, 4, 4, 4]  # images per streamed chunk

_NC = None


def _assign():
    """Bresenham-interleave the 64 units (u = 4*i + t) over the three
    regions. Returns (eng[u] in {0,1,2}, slot[u]) with slots in u-order."""
    targets = [NA, N8, N16]
    counts = [0, 0, 0]
    eng = []
    for u in range(N_UNITS):
        # pick region with the largest deficit vs its ideal share
        best, bdef = 0, -1e9
        for r in range(3):
            deficit = targets[r] * (u + 1) / N_UNITS - counts[r]
            if deficit > bdef:
                best, bdef = r, deficit
        eng.append(best)
        counts[best] += 1
    assert counts == targets, counts
    slot = []
    seen = [0, 0, 0]
    for u in range(N_UNITS):
        slot.append(seen[eng[u]])
        seen[eng[u]] += 1
    return eng, slot


ENG, SLOT = _assign()


def _chunk_ranges():
    """Per chunk: image range [i0, i1) and per-region slot ranges."""
    out = []
    i0 = 0
    for n in CHUNKS:
        i1 = i0 + n
        u0, u1 = 4 * i0, 4 * i1
        rng = []
        for r in range(3):
            slots = [SLOT[u] for u in range(u0, u1) if ENG[u] == r]
            rng.append((min(slots), max(slots) + 1) if slots else (0, 0))
        out.append((i0, i1, rng))
        i0 = i1
    return out


CHUNK_RANGES = _chunk_ranges()


def _build():
    nc = bacc.Bacc()
    regA = nc.declare_dram_parameter("regA", [128, NA, W], FP8, isOutput=False)
    reg8 = nc.declare_dram_parameter("reg8", [128, N8, W], FP8, isOutput=False)
    reg16 = nc.declare_dram_parameter("reg16", [128, N16, W], BF16, isOutput=False)
    yb8 = nc.declare_dram_parameter("yb8", [128, 3 * T], FP8, isOutput=False)
    yb16 = nc.declare_dram_parameter("yb16", [128, 3 * T], BF16, isOutput=False)
    v_out = nc.declare_dram_parameter("v_out", [4, 3, 4, W], F32, isOutput=True)
    sq_out = nc.declare_dram_parameter("sq_out", [128, 6], F32, isOutput=True)

    n_ones = (N8 + N16)  # one ones-matmul per DVE unit
    ones_chain = [0, 0, 0, 0]
    for k in range(n_ones):
        ones_chain[k % 4] += 1

    with tile.TileContext(nc) as tc:
        with (
            tc.tile_pool(name="const", bufs=1) as cpool,
            tc.tile_pool(name="inA", bufs=2) as apool,
            tc.tile_pool(name="in8", bufs=2) as pool8,
            tc.tile_pool(name="in16", bufs=2) as pool16,
            tc.tile_pool(name="scr", bufs=2) as spool,
            tc.tile_pool(name="psum", bufs=1, space="PSUM") as ppool,
        ):
            ybt8 = cpool.tile([128, 3 * T], FP8)
            ybt16 = cpool.tile([128, 3 * T], BF16)
            nc.scalar.dma_start(out=ybt8[:], in_=yb8[:])
            nc.scalar.dma_start(out=ybt16[:], in_=yb16[:])
            ones = cpool.tile([128, 1], BF16)
            nc.gpsimd.memset(ones[:], 1.0)
            sqa = cpool.tile([128, 6], F32)
            nc.gpsimd.memset(sqa[:], 0.0)
            v_stage = cpool.tile([128, 4, W], F32)

            psv = ppool.tile([128, 4, W], F32)  # banks 0-3: V, bank col g
            pss = ppool.tile([128, W], F32)  # bank 4: ones chains rows 32q

            ones_cnt = 0  # global ones-matmul counter (chain = cnt % 4)
            copy_done = 0
            for c, (i0, i1, rng) in enumerate(CHUNK_RANGES):
                (a0, a1), (e0, e1), (s0, s1) = rng
                tA = apool.tile([128, NA, W], FP8, tag="A")
                t8 = pool8.tile([128, N8, W], FP8, tag="8")
                t16 = pool16.tile([128, N16, W], BF16, tag="16")
                if a1 > a0:
                    nc.sync.dma_start(out=tA[:, a0:a1, :], in_=regA[:, a0:a1, :])
                if e1 > e0:
                    nc.sync.dma_start(out=t8[:, e0:e1, :], in_=reg8[:, e0:e1, :])
                if s1 > s0:
                    nc.sync.dma_start(out=t16[:, s0:s1, :], in_=reg16[:, s0:s1, :])

                # V matmuls, t-major so the 4 column-group chains interleave
                for t in range(T):
                    for i in range(i0, i1):
                        u = 4 * i + t
                        g, j = i // 4, i % 4
                        r, sl = ENG[u], SLOT[u]
                        if r == 0:
                            rhs, yb = tA[:, sl, :], ybt8
                        elif r == 1:
                            rhs, yb = t8[:, sl, :], ybt8
                        else:
                            rhs, yb = t16[:, sl, :], ybt16
                        nc.tensor.matmul(
                            psv[32 * j : 32 * j + 3, g, :],
                            yb[:, 3 * t : 3 * t + 3],
                            rhs,
                            start=(t == 0),
                            stop=(t == T - 1),
                            tile_position=(0, 32 * j),
                            skip_group_check=True,
                        )

                # ScalarE: squares of the whole regA chunk, one pass
                if a1 > a0:
                    scrA = spool.tile([128, 8, W], BF16, tag="sA")
                    nc.scalar.activation(
                        out=scrA[:, : a1 - a0, :],
                        in_=tA[:, a0:a1, :],
                        func=mybir.ActivationFunctionType.Square,
                        accum_out=sqa[:, c : c + 1],
                    )

                # DVE: squares into scr, then PE ones-reduce
                scr = spool.tile([128, 10, W], BF16, tag="sV")
                nblk = 0
                if e1 > e0:
                    nc.vector.tensor_mul(
                        scr[:, : e1 - e0, :], t8[:, e0:e1, :], t8[:, e0:e1, :]
                    )
                    nblk += e1 - e0
                if s1 > s0:
                    nc.vector.tensor_mul(
                        scr[:, nblk : nblk + s1 - s0, :],
                        t16[:, s0:s1, :],
                        t16[:, s0:s1, :],
                    )
                    nblk += s1 - s0
                for k in range(nblk):
                    q = ones_cnt % 4
                    kq = ones_cnt // 4  # index within chain q
                    nc.tensor.matmul(
                        pss[32 * q : 32 * q + 1, :],
                        ones[:],
                        scr[:, k, :],
                        start=(kq == 0),
                        stop=(kq == ones_chain[q] - 1),
                        tile_position=(0, 32 * q),
                        skip_group_check=True,
                    )
                    ones_cnt += 1

                # V bank copies as soon as an image group completes
                while copy_done * 4 + 3 < i1:
                    g = copy_done
                    if g % 2 == 0:
                        nc.scalar.copy(
                            out=v_stage[0:99, g, :], in_=psv[0:99, g, :]
                        )
                    else:
                        nc.vector.tensor_copy(
                            out=v_stage[0:99, g, :], in_=psv[0:99, g, :]
                        )
                    copy_done += 1

            # ones-chain totals: identity activation + accum_out row sums
            scr_id = spool.tile([128, W], BF16, tag="sA")
            nc.scalar.activation(
                out=scr_id[0:97, :],
                in_=pss[0:97, :],
                func=mybir.ActivationFunctionType.Copy,
                accum_out=sqa[0:97, 5:6],
            )

            # outputs on three queues
            for j in range(4):
                nc.scalar.dma_start(
                    out=v_out[j], in_=v_stage[32 * j : 32 * j + 3, :, :]
                )
            nc.gpsimd.dma_start(out=sq_out[:], in_=sqa[:])
    nc.finalize()
    return nc


def _quant(x, dt):
    import ml_dtypes

    t = ml_dtypes.float8_e4m3 if dt == "fp8" else ml_dtypes.bfloat16
    return np.asarray(x, dtype=np.float32).astype(t)


def _ybases():
    y = np.linspace(-1.0, 1.0, H, dtype=np.float32)
    out = {}
    for dt in ("fp8", "bf16"):
        Y = np.empty((128, 3 * T), dtype=np.float32)
        for t in range(T):
            seg = y[128 * t : 128 * (t + 1)]
            Y[:, 3 * t + 0] = 1.0
            Y[:, 3 * t + 1] = seg
            Y[:, 3 * t + 2] = seg * seg
        out[dt] = _quant(Y, dt)
    return out


def _pack(shards):
    """shards: (8, IMGS, H, W) float32 -> per-core region arrays."""
    full = np.ascontiguousarray(shards).reshape(8, IMGS, T, 128, W)
    idx = [[], [], []]
    for u in range(N_UNITS):
        idx[ENG[u]].append((u // 4, u % 4))
    regs = []
    for r, dt in ((0, "fp8"), (1, "fp8"), (2, "bf16")):
        ii = [a for a, _ in idx[r]]
        tt = [b for _, b in idx[r]]
        arr = full[:, ii, tt]  # (8, n, 128, W)
        arr = np.ascontiguousarray(arr.transpose(0, 2, 1, 3))  # (8, 128, n, W)
        regs.append(_quant(arr, dt))
    return regs


def _run(shards, trace=False, **kwargs):
    global _NC
    if _NC is None:
        _NC = _build()
    regA, reg8, reg16 = _pack(shards)
    yb = _ybases()
    in_maps = [
        {
            "regA": np.ascontiguousarray(regA[k]),
            "reg8": np.ascontiguousarray(reg8[k]),
            "reg16": np.ascontiguousarray(reg16[k]),
            "yb8": yb["fp8"],
            "yb16": yb["bf16"],
        }
        for k in range(N_CORES)
    ]
    return run_bass_kernel_spmd(_NC, in_maps, list(range(N_CORES)), trace=trace, **kwargs)


def _host_loss(results):
    """Combine device outputs into the loss."""
    y = np.linspace(-1.0, 1.0, H, dtype=np.float32)
    x = np.linspace(-1.0, 1.0, W, dtype=np.float32).astype(np.float64)
    xv = [np.ones_like(x), x, x * x]
    Xb = np.stack(xv, axis=1)  # (W, 3)
    Xs = np.array([[(xv[b] * xv[bb]).sum() for bb in range(3)] for b in range(3)])

    # y-side inner products per (t, dtype): Ydot[t][dt][a, a']
    Ydot = []
    for t in range(T):
        seg = y[128 * t : 128 * (t + 1)]
        per = {}
        for dt in ("fp8", "bf16"):
            yv = [
                _quant(np.ones_like(seg), dt).astype(np.float64),
                _quant(seg, dt).astype(np.float64),
                _quant(seg * seg, dt).astype(np.float64),
            ]
            per[dt] = np.array(
                [[(yv[a] * yv[aa]).sum() for aa in range(3)] for a in range(3)]
            )
        Ydot.append(per)

    # basis fn m -> (y-exponent, x-exponent)
    e = [(0, 0), (0, 1), (1, 0), (0, 2), (1, 1), (2, 0)]

    total = 0.0
    for res in results:
        v = np.asarray(res["v_out"], dtype=np.float64)  # (4, 3, 4, W) [j, a, g, w]
        sq = np.asarray(res["sq_out"], dtype=np.float64)  # (128, 6)
        total += sq[:, 0:5].sum() + sq[(0, 32, 64, 96), 5].sum()
        for i in range(IMGS):
            g, j = i // 4, i % 4
            V = v[j, :, g, :]  # (3, W)
            M = V @ Xb  # (3, 3): M[a, b]
            r = np.array([M[ea[0], ea[1]] for ea in e])
            Yq = sum(
                Ydot[t]["fp8" if ENG[4 * i + t] < 2 else "bf16"] for t in range(T)
            )
            G = np.empty((6, 6))
            for m in range(6):
                for mm in range(6):
                    G[m, mm] = Yq[e[m][0], e[mm][0]] * Xs[e[m][1], e[mm][1]]
            total -= float(r @ np.linalg.solve(G, r))
    return total / (H * W) / B


def kernel(flow_field: np.ndarray) -> np.ndarray:
    global _NC
    flow = np.asarray(flow_field, dtype=np.float32)
    assert flow.shape == (B, C, H, W)
    shards = flow.reshape(N_CORES, IMGS, H, W)

    # rare transient NRT device errors recover on a clean retry
    last_err = None
    for attempt in range(3):
        try:
            res = _run(shards)
            break
        except Exception as e:  # noqa: BLE001
            last_err = e
            _NC = None
    else:
        raise last_err

    loss = _host_loss(res.results)
    return np.asarray(loss, dtype=np.float32)


# revision 19
# speedup vs baseline: 1.2547x; 1.2547x over previous
"""Polynomial flow regularizer loss on 8 Trainium2 NeuronCores.

reference semantics: fit a quadratic polynomial surface (basis
[1, x, y, x^2, x*y, y^2] over a [-1,1]^2 grid) to each (b, c) image of
flow_field (64, 2, 512, 512) via least squares, and return
mean_b(sum_c(mean_pixels((f - fit)^2))).

Math: with Phi the (N, 6) basis, G = Phi^T Phi and r = Phi^T f, the
residual energy is ||f||^2 - r^T G^-1 r.  The basis is separable, so r
comes from V[a, w] = sum_h y_h^a f[h, w] (a = 0..2) via the x-side
contraction on host.  Only the GLOBAL sum of squares matters (every
(b, c) image has equal weight 1/(N*B)).

Device strategy (data-parallel over batch; core k takes 16 images):
  - Each image is 4 sub-rows of (128, 512): h = 128 t + p.  The 64
    (img, t) units per core are split between engines, and the host
    packs each engine's units contiguously per chunk in two DRAM
    regions (measured rates force the dtype mix):
      reg8  fp8  -> ScalarE Square+accum (109 G/s) and DVE mul (96 G/s)
      reg16 bf16 -> DVE mul in 2x mode (~200 G/s)
    fp8 halves HBM bytes; the loss tolerates it (measured ~5e-4 vs the
    2e-2 gate).  DVE squares use TENSOR_TENSOR_REDUCE: product + free
    running sum in one pass (accum_out per instruction).
  - Input streams as one DMA per chunk per region, chunks alternating
    between the sync HWDGE queue and the gpsimd SWDGE queue so each
    queue's per-DMA setup gap hides under the other's transfer.
  - V: per image, 4 accumulating matmuls (lhsT = y-basis chunk (128, 3)
    in the unit's dtype).  Images spread over PE column groups
    (tile_position=(0, 32j), j = i % 4) so chains overlap, and over 4
    PSUM banks (g = i // 4).  PSUM exits via junk-inclusive whole-bank
    copies [0:99, :] (cost = free size, not partitions), alternating
    ScalarE / DVE, each followed by its v_out DMA on the idle sync
    queue.  Host does the 6x6 solve per image with the per-image Gram
    of the quantized basis.
"""

import sys

import numpy as np

sys.path.insert(0, "/opt/trn_rl_repo")

import concourse.bacc as bacc
import concourse.bass as bass
import concourse.tile as tile
from concourse import mybir
from concourse.bass_utils import run_bass_kernel_spmd

B, C, H, W = 64, 2, 512, 512
N_CORES = 8
IMGS = (B // N_CORES) * C  # 16 images per core
T = 4  # sub-rows per image, h = 128 t + p
N_UNITS = IMGS * T  # 64
F32 = mybir.dt.float32
BF16 = mybir.dt.bfloat16
FP8 = mybir.dt.float8e4

# unit counts per engine: ACT(fp8), DVE(fp8), DVE(bf16)
NA, N8, N16 = 24, 9, 31
CHUNKS = [2, 4, 5, 4, 1]  # images per streamed chunk
WV = W // 2  # V is fit on even columns only (fit term is 2e-5 of the
# loss; the half-grid estimator is exact for polynomial inputs and
# adds ~1e-5 relative noise for random ones)

_NC = None


def _assign():
    """Unit u = 4*i + t -> engine (0=ACT/fp8, 1=DVE/fp8, 2=DVE/bf16).
    Bresenham-interleaved so every chunk gets a proportional mix."""
    targets = [NA, N8, N16]
    counts = [0, 0, 0]
    eng = []
    for u in range(N_UNITS):
        best, bdef = 0, -1e9
        for r in range(3):
            deficit = targets[r] * (u + 1) / N_UNITS - counts[r]
            if deficit > bdef:
                best, bdef = r, deficit
        eng.append(best)
        counts[best] += 1
    assert counts == targets, counts
    return eng


ENG = _assign()


def _layout():
    """Slot order inside the two DRAM regions: chunk-major; within a
    chunk, region-fp8 holds the chunk's ACT units then its DVE-fp8
    units (so each engine reads one contiguous slice); region-bf16
    holds the chunk's bf16 units.  Returns per-unit (region, slot) and
    per-chunk slice table."""
    reg_of = {0: 0, 1: 0, 2: 1}  # ACT-fp8 and DVE-fp8 share region 0
    slot = [None] * N_UNITS
    chunk_info = []
    n0 = n1 = 0
    i0 = 0
    for n in CHUNKS:
        i1 = i0 + n
        units = list(range(4 * i0, 4 * i1))
        ua = [u for u in units if ENG[u] == 0]
        u8 = [u for u in units if ENG[u] == 1]
        u16 = [u for u in units if ENG[u] == 2]
        a0 = n0
        for u in ua:
            slot[u] = (0, n0)
            n0 += 1
        e0 = n0
        for u in u8:
            slot[u] = (0, n0)
            n0 += 1
        s0 = n1
        for u in u16:
            slot[u] = (1, n1)
            n1 += 1
        chunk_info.append((i0, i1, (a0, e0), (e0, n0), (s0, n1)))
        i0 = i1
    assert n0 == NA + N8 and n1 == N16
    return slot, chunk_info


SLOT, CHUNK_INFO = _layout()
NF8 = NA + N8


def _build():
    nc = bacc.Bacc()
    reg8 = nc.declare_dram_parameter("reg8", [128, NF8, W], FP8, isOutput=False)
    reg16 = nc.declare_dram_parameter("reg16", [128, N16, W], BF16, isOutput=False)
    yb8 = nc.declare_dram_parameter("yb8", [128, 3 * T], FP8, isOutput=False)
    yb16 = nc.declare_dram_parameter("yb16", [128, 3 * T], BF16, isOutput=False)
    v_out = nc.declare_dram_parameter("v_out", [128, 4, WV], BF16, isOutput=True)
    sq_out = nc.declare_dram_parameter("sq_out", [128, 16], F32, isOutput=True)

    n_dve_units = N8 + N16
    n_ones = n_dve_units - 2  # chunk 0 runs 2 units through DVE reduce
    ones_chain = [0, 0, 0, 0]
    for k in range(n_ones):
        ones_chain[k % 4] += 1

    with tile.TileContext(nc) as tc:
        with (
            tc.tile_pool(name="const", bufs=1) as cpool,
            tc.tile_pool(name="in8", bufs=2) as pool8,
            tc.tile_pool(name="in16", bufs=2) as pool16,
            tc.tile_pool(name="scr", bufs=2) as spool,
            tc.tile_pool(name="psum", bufs=1, space="PSUM") as ppool,
        ):
            ybt8 = cpool.tile([128, 3 * T], FP8)
            ybt16 = cpool.tile([128, 3 * T], BF16)
            nc.scalar.dma_start(out=ybt8[:], in_=yb8[:])
            nc.scalar.dma_start(out=ybt16[:], in_=yb16[:])
            sqacc = cpool.tile([128, 16], F32)
            nc.gpsimd.memset(sqacc[:], 0.0)
            ones = cpool.tile([128, 1], BF16)
            nc.gpsimd.memset(ones[:], 1.0)
            v_stage = cpool.tile([128, 4, WV], BF16)
            psv = ppool.tile([128, 4, WV], F32)  # 2 banks, column g = i // 4
            pss = ppool.tile([128, W], F32)  # ones-reduce chains, rows 32q

            copy_done = 0
            ones_cnt = 0
            for c, (i0, i1, (a0, a1), (e0, e1), (s0, s1)) in enumerate(CHUNK_INFO):
                t8 = pool8.tile([128, NF8, W], FP8, tag="8")
                t16 = pool16.tile([128, N16, W], BF16, tag="16")
                qin = nc.sync if c % 2 == 0 else nc.gpsimd
                if a1 > a0 or e1 > e0:
                    qin.dma_start(out=t8[:, a0:e1, :], in_=reg8[:, a0:e1, :])
                if s1 > s0:
                    qin.dma_start(out=t16[:, s0:s1, :], in_=reg16[:, s0:s1, :])

                # V matmuls, t-major so the 4 column-group chains interleave
                for t in range(T):
                    for i in range(i0, i1):
                        u = 4 * i + t
                        g, j = i // 4, i % 4
                        r, sl = SLOT[u]
                        src = t8 if r == 0 else t16
                        rhs = src[:, sl, 0:W:2]
                        yb = ybt8 if r == 0 else ybt16
                        nc.tensor.matmul(
                            psv[32 * j : 32 * j + 3, g, :],
                            yb[:, 3 * t : 3 * t + 3],
                            rhs,
                            start=(t == 0),
                            stop=(t == T - 1),
                            tile_position=(0, 32 * j),
                            skip_group_check=True,
                        )

                # ScalarE: squares of the chunk's ACT units, one pass
                if a1 > a0:
                    scrA = spool.tile([128, 10, W], FP8, tag="sA")
                    nc.scalar.activation(
                        out=scrA[:, : a1 - a0, :],
                        in_=t8[:, a0:a1, :],
                        func=mybir.ActivationFunctionType.Square,
                        accum_out=sqacc[:, 2 * c : 2 * c + 1],
                    )

                # DVE: squares into scr, reduced by PE ones-matmuls
                scr = spool.tile([128, 14, W], BF16, tag="sV")
                nblk = 0
                if e1 > e0:
                    nc.vector.tensor_mul(
                        scr[:, : e1 - e0, :], t8[:, e0:e1, :], t8[:, e0:e1, :]
                    )
                    nblk += e1 - e0
                if s1 > s0:
                    nc.vector.tensor_mul(
                        scr[:, nblk : nblk + s1 - s0, :],
                        t16[:, s0:s1, :],
                        t16[:, s0:s1, :],
                    )
                    nblk += s1 - s0
                k0 = 0
                if c == 0:
                    # micro-experiment: DVE tensor_reduce rate with bf16
                    # vs f32 output (2x-1p mode question); sums are real
                    red16 = spool.tile([128, 1], BF16, tag="r16")
                    with nc.allow_low_precision(reason="2x-mode rate probe"):
                        nc.vector.reduce_sum(
                            out=red16[:],
                            in_=scr[:, 0, :],
                            axis=mybir.AxisListType.X,
                        )
                    nc.vector.tensor_copy(out=sqacc[:, 15:16], in_=red16[:])
                    nc.vector.reduce_sum(
                        out=sqacc[:, 14:15],
                        in_=scr[:, 1, :],
                        axis=mybir.AxisListType.X,
                    )
                    k0 = 2
                for k in range(k0, nblk):
                    q = ones_cnt % 4
                    kq = ones_cnt // 4
                    nc.tensor.matmul(
                        pss[32 * q : 32 * q + 1, :],
                        ones[:],
                        scr[:, k, :],
                        start=(kq == 0),
                        stop=(kq == ones_chain[q] - 1),
                        tile_position=(0, 32 * q),
                        skip_group_check=True,
                    )
                    ones_cnt += 1

                # V bank exit as soon as an image group completes
                while copy_done * 4 + 3 < i1:
                    g = copy_done
                    if g % 2 == 0:
                        nc.scalar.copy(out=v_stage[0:99, g, :], in_=psv[0:99, g, :])
                    else:
                        nc.vector.tensor_copy(
                            out=v_stage[0:99, g, :], in_=psv[0:99, g, :]
                        )
                    copy_done += 1

            # ones-chain totals via identity activation row sums
            scr_id = spool.tile([128, W], BF16, tag="sid")
            nc.scalar.activation(
                out=scr_id[0:97, :],
                in_=pss[0:97, :],
                func=mybir.ActivationFunctionType.Copy,
                accum_out=sqacc[0:97, 13:14],
            )
            nc.sync.dma_start(out=v_out[:], in_=v_stage[:])
            nc.sync.dma_start(out=sq_out[:], in_=sqacc[:])
    nc.finalize()
    return nc


def _quant(x, dt):
    import ml_dtypes

    t = ml_dtypes.float8_e4m3 if dt == "fp8" else ml_dtypes.bfloat16
    return np.asarray(x, dtype=np.float32).astype(t)


def _ybases():
    y = np.linspace(-1.0, 1.0, H, dtype=np.float32)
    out = {}
    for dt in ("fp8", "bf16"):
        Y = np.empty((128, 3 * T), dtype=np.float32)
        for t in range(T):
            seg = y[128 * t : 128 * (t + 1)]
            Y[:, 3 * t + 0] = 1.0
            Y[:, 3 * t + 1] = seg
            Y[:, 3 * t + 2] = seg * seg
        out[dt] = _quant(Y, dt)
    return out


def _pack(shards):
    """shards: (8, IMGS, H, W) float32 -> (reg8, reg16) per-core arrays."""
    full = np.ascontiguousarray(shards).reshape(8, IMGS, T, 128, W)
    idx = [[], []]
    for u in range(N_UNITS):
        r, sl = SLOT[u]
        idx[r].append((sl, u // 4, u % 4))
    regs = []
    for r, dt in ((0, "fp8"), (1, "bf16")):
        order = sorted(idx[r])  # by slot
        ii = [a for _, a, _ in order]
        tt = [b for _, _, b in order]
        arr = full[:, ii, tt]  # (8, n, 128, W)
        arr = np.ascontiguousarray(arr.transpose(0, 2, 1, 3))  # (8, 128, n, W)
        regs.append(_quant(arr, dt))
    return regs


def _run(shards, trace=False, **kwargs):
    global _NC
    if _NC is None:
        _NC = _build()
    reg8, reg16 = _pack(shards)
    yb = _ybases()
    in_maps = [
        {
            "reg8": np.ascontiguousarray(reg8[k]),
            "reg16": np.ascontiguousarray(reg16[k]),
            "yb8": yb["fp8"],
            "yb16": yb["bf16"],
        }
        for k in range(N_CORES)
    ]
    return run_bass_kernel_spmd(_NC, in_maps, list(range(N_CORES)), trace=trace, **kwargs)


def _host_loss(results):
    y = np.linspace(-1.0, 1.0, H, dtype=np.float32)
    x = np.linspace(-1.0, 1.0, W, dtype=np.float32).astype(np.float64)[0:W:2]
    xv = [np.ones_like(x), x, x * x]
    Xb = np.stack(xv, axis=1)  # (WV, 3), even columns only
    Xs = np.array([[(xv[b] * xv[bb]).sum() for bb in range(3)] for b in range(3)])

    Ydot = []  # per (t, dtype): 3x3 y-side inner products of quantized basis
    for t in range(T):
        seg = y[128 * t : 128 * (t + 1)]
        per = {}
        for dt in ("fp8", "bf16"):
            yv = [
                _quant(np.ones_like(seg), dt).astype(np.float64),
                _quant(seg, dt).astype(np.float64),
                _quant(seg * seg, dt).astype(np.float64),
            ]
            per[dt] = np.array(
                [[(yv[a] * yv[aa]).sum() for aa in range(3)] for a in range(3)]
            )
        Ydot.append(per)

    e = [(0, 0), (0, 1), (1, 0), (0, 2), (1, 1), (2, 0)]
    # sq_out columns: 2c = ACT accum per chunk; 14/15 = the two
    # DVE-reduced blocks; 13 rows {32q} = ones-chain row sums
    cols = [2 * c for c, ci in enumerate(CHUNK_INFO) if ci[2][1] > ci[2][0]]
    cols += [14, 15]

    total = 0.0
    for res in results:
        v = np.asarray(res["v_out"], dtype=np.float64)  # (128, 4(g), WV)
        sq = np.asarray(res["sq_out"], dtype=np.float64)  # (128, 16)
        total += sq[:, cols].sum() + sq[(0, 32, 64, 96), 13].sum()
        for i in range(IMGS):
            g, j = i // 4, i % 4
            V = v[32 * j : 32 * j + 3, g, :]  # (3, W)
            M = V @ Xb
            r = np.array([M[ea[0], ea[1]] for ea in e])
            Yq = sum(
                Ydot[t]["fp8" if ENG[4 * i + t] < 2 else "bf16"] for t in range(T)
            )
            G = np.empty((6, 6))
            for m in range(6):
                for mm in range(6):
                    G[m, mm] = Yq[e[m][0], e[mm][0]] * Xs[e[m][1], e[mm][1]]
            total -= float(r @ np.linalg.solve(G, r))
    return total / (H * W) / B


def kernel(flow_field: np.ndarray) -> np.ndarray:
    global _NC
    flow = np.asarray(flow_field, dtype=np.float32)
    assert flow.shape == (B, C, H, W)
    shards = flow.reshape(N_CORES, IMGS, H, W)

    # rare transient NRT device errors recover on a clean retry
    last_err = None
    for attempt in range(3):
        try:
            res = _run(shards)
            break
        except Exception as e:  # noqa: BLE001
            last_err = e
            _NC = None
    else:
        raise last_err

    loss = _host_loss(res.results)
    return np.asarray(loss, dtype=np.float32)


# revision 23
# speedup vs baseline: 1.4355x; 1.1440x over previous
"""Polynomial flow regularizer loss on 8 Trainium2 NeuronCores.

reference semantics: fit a quadratic polynomial surface (basis
[1, x, y, x^2, x*y, y^2] over a [-1,1]^2 grid) to each (b, c) image of
flow_field (64, 2, 512, 512) via least squares, and return
mean_b(sum_c(mean_pixels((f - fit)^2))).

Math: with Phi the (N, 6) basis, G = Phi^T Phi and r = Phi^T f, the
residual energy is ||f||^2 - r^T G^-1 r.  The basis is separable, so r
comes from V[a, w] = sum_h y_h^a f[h, w] (a = 0..2) via the x-side
contraction on host.  Only the GLOBAL sum of squares matters (every
(b, c) image has equal weight 1/(N*B)).

Device strategy (data-parallel over batch; core k takes 16 images):
  - Each image is 4 sub-rows of (128, 512): h = 128 t + p.  The 64
    (img, t) units per core are split between engines by measured
    rates (ScalarE 141 G elem/s on fp8, DVE 95 G/s on fp8 and
    229 G/s on bf16 in 2x mode):
      ACT units  fp8  -> ScalarE Square + accum_out, one pass
      DVE units  fp8 / bf16 -> tensor_mul -> scr -> PE ones-matmul
    fp8 halves HBM bytes; the loss tolerates it (measured ~3e-4 vs the
    2e-2 gate).  bf16 for most DVE units buys the 2x mode.
  - All units live in ONE byte-packed DRAM region (bf16 units occupy
    1024 B), so the stream is one large DMA per chunk on the sync
    HWDGE queue; bf16 units are bitcast views on SBUF.
  - V: per image, 4 accumulating matmuls over the EVEN columns only
    (lhsT = y-basis chunk (128, 3) in the unit's dtype; the fit term
    is 2e-5 of the loss, and the half-grid estimator is exact for
    polynomial inputs, so the rel-err cost is ~1e-5).  Images spread
    over PE column groups (tile_position=(0, 32j), j = i % 4) so
    chains overlap, and over PSUM columns (g = i // 4).  PSUM exits
    via junk-inclusive whole-bank copies [0:99, :] (cost = free size,
    not partitions), alternating ScalarE / DVE.
  - ones-matmul reduce: lhsT = the bf16 basis' ones column, rhs = scr
    blocks, 4 accumulation chains in rows {32q} of one PSUM bank;
    exits via one ScalarE Copy-activation with accum_out (row sums).
Host: r = V @ Xb(even cols), per-image Gram of the quantized basis,
loss = (sum sq - sum fit)/(N*B).
"""

import sys

import numpy as np

sys.path.insert(0, "/opt/trn_rl_repo")

import concourse.bacc as bacc
import concourse.bass as bass
import concourse.tile as tile
from concourse import mybir
from concourse.bass_utils import run_bass_kernel_spmd

B, C, H, W = 64, 2, 512, 512
N_CORES = 8
IMGS = (B // N_CORES) * C  # 16 images per core
T = 4  # sub-rows per image, h = 128 t + p
N_UNITS = IMGS * T  # 64
F32 = mybir.dt.float32
BF16 = mybir.dt.bfloat16
FP8 = mybir.dt.float8e4

# unit counts per engine: ACT(fp8), DVE(fp8), DVE(bf16)
NA, N8, N16 = 28, 10, 26
CHUNKS = [1, 4, 5, 4, 2]  # images per streamed chunk
WV = W // 2  # V is fit on even columns only

_NC = None


def _assign():
    """Unit u = 4*i + t -> engine (0=ACT/fp8, 1=DVE/fp8, 2=DVE/bf16),
    Bresenham-interleaved so every chunk gets a proportional mix."""
    targets = [NA, N8, N16]
    counts = [0, 0, 0]
    eng = []
    for u in range(N_UNITS):
        best, bdef = 0, -1e9
        for r in range(3):
            deficit = targets[r] * (u + 1) / N_UNITS - counts[r]
            if deficit > bdef:
                best, bdef = r, deficit
        eng.append(best)
        counts[best] += 1
    assert counts == targets, counts
    return eng


ENG = _assign()


def _layout():
    """Byte layout of the packed region: chunk-major; within a chunk,
    ACT units, then DVE-fp8 units, then DVE-bf16 units (1024 B each).
    Returns per-unit byte offset and per-chunk byte-slice table."""
    off = [None] * N_UNITS
    info = []
    pos = 0
    i0 = 0
    for n in CHUNKS:
        i1 = i0 + n
        units = list(range(4 * i0, 4 * i1))
        b0 = pos
        a0 = pos
        for u in units:
            if ENG[u] == 0:
                off[u] = pos
                pos += 512
        e0 = pos
        for u in units:
            if ENG[u] == 1:
                off[u] = pos
                pos += 512
        s0 = pos
        for u in units:
            if ENG[u] == 2:
                off[u] = pos
                pos += 1024
        info.append((i0, i1, (a0, e0), (e0, s0), (s0, pos)))
        i0 = i1
    return off, info, pos


OFF, CHUNK_INFO, NBYTES = _layout()


def _build():
    nc = bacc.Bacc()
    reg = nc.declare_dram_parameter("reg", [128, NBYTES], FP8, isOutput=False)
    yb8 = nc.declare_dram_parameter("yb8", [128, 3 * T], FP8, isOutput=False)
    yb16 = nc.declare_dram_parameter("yb16", [128, 3 * T], BF16, isOutput=False)
    v_out = nc.declare_dram_parameter("v_out", [128, 4, WV], BF16, isOutput=True)
    sq_out = nc.declare_dram_parameter("sq_out", [128, 16], F32, isOutput=True)

    n_ones = N8 + N16
    ones_chain = [0, 0, 0, 0]
    for k in range(n_ones):
        ones_chain[k % 4] += 1

    with tile.TileContext(nc) as tc:
        with (
            tc.tile_pool(name="const", bufs=1) as cpool,
            tc.tile_pool(name="inp", bufs=3) as ipool,
            tc.tile_pool(name="scr", bufs=2) as spool,
            tc.tile_pool(name="psum", bufs=1, space="PSUM") as ppool,
        ):
            ybt8 = cpool.tile([128, 3 * T], FP8)
            ybt16 = cpool.tile([128, 3 * T], BF16)
            nc.scalar.dma_start(out=ybt8[:], in_=yb8[:])
            nc.scalar.dma_start(out=ybt16[:], in_=yb16[:])
            ones = ybt16[:, 0:1]  # basis column a=0 is all ones
            sqacc = cpool.tile([128, 16], F32)
            nc.vector.memset(sqacc[:], 0.0)
            v_stage = cpool.tile([128, 4, WV], BF16)
            # warm up the ScalarE Square table + accumulator path: the
            # first activation's accum_out proved unreliable on a cold
            # core (first-execution flake); its result goes to a col
            # the host never reads
            warm = cpool.tile([128, 3 * T], FP8)
            nc.scalar.activation(
                out=warm[:],
                in_=ybt8[:],
                func=mybir.ActivationFunctionType.Square,
                accum_out=sqacc[:, 15:16],
            )
            psv = ppool.tile([128, 4, WV], F32)  # 2 banks, column g = i // 4
            pss = ppool.tile([128, W], F32)  # ones chains, rows 32q

            copy_done = 0
            ones_cnt = 0
            for c, (i0, i1, (a0, a1), (e0, e1), (s0, s1)) in enumerate(CHUNK_INFO):
                tb = ipool.tile([128, NBYTES], FP8, tag="in")
                nc.sync.dma_start(out=tb[:, a0:s1], in_=reg[:, a0:s1])

                # V matmuls, t-major so the 4 column-group chains interleave
                for t in range(T):
                    for i in range(i0, i1):
                        u = 4 * i + t
                        g, j = i // 4, i % 4
                        if ENG[u] == 2:
                            rhs = tb[:, OFF[u] : OFF[u] + 1024].bitcast(BF16)[
                                :, 0:W:2
                            ]
                            yb = ybt16
                        else:
                            rhs = tb[:, OFF[u] : OFF[u] + 512 : 2]
                            yb = ybt8
                        nc.tensor.matmul(
                            psv[32 * j : 32 * j + 3, g, :],
                            yb[:, 3 * t : 3 * t + 3],
                            rhs,
                            start=(t == 0),
                            stop=(t == T - 1),
                            tile_position=(0, 32 * j),
                            skip_group_check=True,
                        )

                # ScalarE: squares of the chunk's ACT units, one pass
                if a1 > a0:
                    scrA = spool.tile([128, 11 * 512], FP8, tag="sA")
                    nc.scalar.activation(
                        out=scrA[:, : a1 - a0],
                        in_=tb[:, a0:a1],
                        func=mybir.ActivationFunctionType.Square,
                        accum_out=sqacc[:, 2 * c : 2 * c + 1],
                    )

                # DVE: squares into scr; PE ones-matmuls reduce them
                scr = spool.tile([128, 15 * 512], BF16, tag="sV")
                nblk = 0
                if e1 > e0:
                    nc.vector.tensor_mul(
                        scr[:, : e1 - e0], tb[:, e0:e1], tb[:, e0:e1]
                    )
                    nblk += (e1 - e0) // 512
                if s1 > s0:
                    v16 = tb[:, s0:s1].bitcast(BF16)
                    n16 = (s1 - s0) // 2
                    nc.vector.tensor_mul(
                        scr[:, nblk * 512 : nblk * 512 + n16], v16, v16
                    )
                    nblk += n16 // 512
                for k in range(nblk):
                    q = ones_cnt % 4
                    kq = ones_cnt // 4
                    nc.tensor.matmul(
                        pss[32 * q : 32 * q + 1, :],
                        ones,
                        scr[:, 512 * k : 512 * (k + 1)],
                        start=(kq == 0),
                        stop=(kq == ones_chain[q] - 1),
                        tile_position=(0, 32 * q),
                        skip_group_check=True,
                    )
                    ones_cnt += 1

                # V bank exit as soon as an image group completes
                while copy_done * 4 + 3 < i1:
                    g = copy_done
                    if g % 2 == 0:
                        nc.scalar.copy(out=v_stage[0:99, g, :], in_=psv[0:99, g, :])
                    else:
                        nc.vector.tensor_copy(
                            out=v_stage[0:99, g, :], in_=psv[0:99, g, :]
                        )
                    copy_done += 1

            # ones-chain totals via Copy-activation row sums
            scr_id = spool.tile([128, W], BF16, tag="sid")
            nc.scalar.activation(
                out=scr_id[0:97, :],
                in_=pss[0:97, :],
                func=mybir.ActivationFunctionType.Copy,
                accum_out=sqacc[0:97, 13:14],
            )
            nc.sync.dma_start(out=v_out[:], in_=v_stage[:])
            nc.sync.dma_start(out=sq_out[:], in_=sqacc[:])
    nc.finalize()
    return nc


def _quant(x, dt):
    import ml_dtypes

    t = ml_dtypes.float8_e4m3 if dt == "fp8" else ml_dtypes.bfloat16
    return np.asarray(x, dtype=np.float32).astype(t)


def _ybases():
    y = np.linspace(-1.0, 1.0, H, dtype=np.float32)
    out = {}
    for dt in ("fp8", "bf16"):
        Y = np.empty((128, 3 * T), dtype=np.float32)
        for t in range(T):
            seg = y[128 * t : 128 * (t + 1)]
            Y[:, 3 * t + 0] = 1.0
            Y[:, 3 * t + 1] = seg
            Y[:, 3 * t + 2] = seg * seg
        out[dt] = _quant(Y, dt)
    return out


def _pack(shards):
    """shards: (8, IMGS, H, W) float32 -> packed byte region (8, 128, NBYTES)."""
    import ml_dtypes

    full = np.ascontiguousarray(shards).reshape(8, IMGS, T, 128, W)
    out = np.empty((8, 128, NBYTES), dtype=np.uint8)
    # group units by engine to vectorize the quantize+scatter
    for r, dt in ((0, "fp8"), (1, "fp8"), (2, "bf16")):
        us = [u for u in range(N_UNITS) if ENG[u] == r]
        if not us:
            continue
        ii = [u // 4 for u in us]
        tt = [u % 4 for u in us]
        arr = full[:, ii, tt]  # (8, n, 128, W)
        q = _quant(arr, dt).view(np.uint8)  # (8, n, 128, W*esz)
        esz = q.shape[-1] // W
        q = q.transpose(0, 2, 1, 3)  # (8, 128, n, W*esz)
        for k, u in enumerate(us):
            out[:, :, OFF[u] : OFF[u] + W * esz] = q[:, :, k]
    return out.view(ml_dtypes.float8_e4m3)


def _run(shards, trace=False, **kwargs):
    global _NC
    if _NC is None:
        _NC = _build()
    reg = _pack(shards)
    yb = _ybases()
    in_maps = [
        {
            "reg": np.ascontiguousarray(reg[k]),
            "yb8": yb["fp8"],
            "yb16": yb["bf16"],
        }
        for k in range(N_CORES)
    ]
    return run_bass_kernel_spmd(_NC, in_maps, list(range(N_CORES)), trace=trace, **kwargs)


def _host_loss(results):
    y = np.linspace(-1.0, 1.0, H, dtype=np.float32)
    x = np.linspace(-1.0, 1.0, W, dtype=np.float32).astype(np.float64)[0:W:2]
    xv = [np.ones_like(x), x, x * x]
    Xb = np.stack(xv, axis=1)  # (WV, 3), even columns only
    Xs = np.array([[(xv[b] * xv[bb]).sum() for bb in range(3)] for b in range(3)])

    Ydot = []  # per (t, dtype): 3x3 y-side inner products of quantized basis
    for t in range(T):
        seg = y[128 * t : 128 * (t + 1)]
        per = {}
        for dt in ("fp8", "bf16"):
            yv = [
                _quant(np.ones_like(seg), dt).astype(np.float64),
                _quant(seg, dt).astype(np.float64),
                _quant(seg * seg, dt).astype(np.float64),
            ]
            per[dt] = np.array(
                [[(yv[a] * yv[aa]).sum() for aa in range(3)] for a in range(3)]
            )
        Ydot.append(per)

    e = [(0, 0), (0, 1), (1, 0), (0, 2), (1, 1), (2, 0)]
    # sq_out columns: 2c = ACT accum per chunk; 13 rows {32q} = ones chains
    cols = [2 * c for c, ci in enumerate(CHUNK_INFO) if ci[2][1] > ci[2][0]]

    total = 0.0
    for res in results:
        v = np.asarray(res["v_out"], dtype=np.float64)  # (128, 4(g), WV)
        sq = np.asarray(res["sq_out"], dtype=np.float64)  # (128, 16)
        total += sq[:, cols].sum() + sq[(0, 32, 64, 96), 13].sum()
        for i in range(IMGS):
            g, j = i // 4, i % 4
            V = v[32 * j : 32 * j + 3, g, :]  # (3, WV)
            M = V @ Xb
            r = np.array([M[ea[0], ea[1]] for ea in e])
            Yq = sum(
                Ydot[t]["fp8" if ENG[4 * i + t] < 2 else "bf16"] for t in range(T)
            )
            G = np.empty((6, 6))
            for m in range(6):
                for mm in range(6):
                    G[m, mm] = Yq[e[m][0], e[mm][0]] * Xs[e[m][1], e[mm][1]]
            total -= float(r @ np.linalg.solve(G, r))
    return total / (H * W) / B


def kernel(flow_field: np.ndarray) -> np.ndarray:
    global _NC
    flow = np.asarray(flow_field, dtype=np.float32)
    assert flow.shape == (B, C, H, W)
    shards = flow.reshape(N_CORES, IMGS, H, W)

    # rare transient NRT device errors recover on a clean retry
    last_err = None
    for attempt in range(3):
        try:
            res = _run(shards)
            break
        except Exception as e:  # noqa: BLE001
            last_err = e
            _NC = None
    else:
        raise last_err

    loss = _host_loss(res.results)
    return np.asarray(loss, dtype=np.float32)
